# revision 9
# baseline (speedup 1.0000x reference)
"""Trainium2 kernel for nn_HATGNN: hierarchical label<-patch kNN aggregation.

The 99.9%-of-FLOPs part (832x100000 squared-euclidean cdist + top-9
selection) runs on 8 NeuronCores, patch-sharded (12500 rows/core).

Device-side design (per core):
- Scores z2[s,j] ~= 4L.s @ p_j - 2|p_j|^2 are computed with one fp8e4m3 DoubleRow
  matmul per 512-column tile (K=256 channels in a single pass).
  Operands are quantized to coarse integer grids (labels: round(4l) ints,
  patches: 0.5-grid) so every product lands on a 0.5 grid and the PSUM
  accumulation is EXACT (score magnitude < 2^11, grid 2^-13: 24 bits).
- 6 of the 256 channels are repurposed as aux channels folding in
  (a) -2|p_j|^2 (2 channels, error <= 1 scaled) and (b) the column index
  within its 2048-wide selection window, encoded in mantissa bits
  2^-5..2^-13 (4 channels, exact).  A single fp32 score carries both.
- The ONLY selection work is one Max8 per 2048-wide (4 PSUM banks)
  window: 7 windows x 7 label chunks per core.  No FIND_INDEX8 pass, no
  |p|^2 rank-1 matmul, no index DMA.
- The host decodes (score, column) from the fp32 values, merges the
  8-core union (448 candidates/label), rescores candidates EXACTLY and
  takes the true top-9.  Quantization noise (sigma ~4 on a d^2 scale
  where the union-miss margin is ~40) only affects which candidates
  enter the union, not the final ordering.  The 6 dropped data channels
  are aligned with the label matrix's least-energy singular directions.
- The tiny 3-level MLP/LayerNorm pipeline (<=832 rows) runs in numpy.
"""
import numpy as np
import ml_dtypes

import concourse.bacc as bacc
import concourse.mybir as mybir
from concourse.tile import TileContext
from concourse.bass_utils import run_bass_kernel_spmd

NCORES = 8
NPER = 12500          # patches per core (no padding)
TW = 512              # matmul tile width (one PSUM bank)
WIN = 2048            # selection window (4 PSUM banks)
NWIN = 7              # 6 x 2048 + 1 x 212
SPLIT1 = 2048         # patch SBUF tiles: window 0 | tiles 4-12 | tiles 13-24
SPLIT2 = 6656
S = 832               # total labels (64 mood + 256 genre + 512 sub)
SL = 896              # padded to 7 x 128
NCHUNK = SL // 128    # 7 label chunks
C = 256
D = 250               # data channels (6 aux)
NCAND = NWIN * 8      # 56 candidates per label per core
EPS = 1e-5

F8 = mybir.dt.float8e4
F32 = mybir.dt.float32
PC_PAD = 5856         # pC SBUF tile cols (NPER-SPLIT2=5844 padded to %16==0
                      # for the DoubleRow ko-stride; last 12 cols never read)

_CACHE = {}
LAST_RESULT = None    # BassKernelResults of the most recent device run


def _build_nc():
    nc = bacc.Bacc()
    labT = nc.dram_tensor("labT", [128, 2, SL], F8, kind="ExternalInput")
    patA = nc.dram_tensor("patA", [128, 2, SPLIT1], F8, kind="ExternalInput")
    patB = nc.dram_tensor("patB", [128, 2, SPLIT2 - SPLIT1], F8,
                          kind="ExternalInput")
    patC = nc.dram_tensor("patC", [128, 2, NPER - SPLIT2], F8,
                          kind="ExternalInput")
    cand = nc.dram_tensor("cand", [SL, NCAND], F32, kind="ExternalOutput")

    DR = mybir.MatmulPerfMode.DoubleRow

    with TileContext(nc) as tc:
        with tc.tile_pool(name="big", bufs=1) as bigp, \
             tc.tile_pool(name="work", bufs=NCHUNK) as workp, \
             tc.tile_pool(name="ps", bufs=2, space="PSUM") as psp:
            lab_t = bigp.tile([128, 2, SL], F8, tag="lab")
            pA_t = bigp.tile([128, 2, SPLIT1], F8, tag="pA")
            pB_t = bigp.tile([128, 2, SPLIT2 - SPLIT1], F8, tag="pB")
            pC_t = bigp.tile([128, 2, PC_PAD], F8, tag="pC")
            # spread input DMAs across idle engine queues so they land in
            # parallel (~4us instead of ~10us serial); Vector/Tensor queues
            # stay clean for the hot loop.
            nc.sync.dma_start(out=lab_t[:], in_=labT[:])
            nc.gpsimd.dma_start(out=pA_t[:], in_=patA[:])
            nc.scalar.dma_start(out=pB_t[:], in_=patB[:])
            nc.sync.dma_start(out=pC_t[:, :, :NPER - SPLIT2], in_=patC[:])

            def rhs_slice(c0, w):
                if c0 < SPLIT1:
                    return pA_t[:, :, c0:c0 + w]
                if c0 < SPLIT2:
                    return pB_t[:, :, c0 - SPLIT1:c0 - SPLIT1 + w]
                return pC_t[:, :, c0 - SPLIT2:c0 - SPLIT2 + w]

            for lc in range(NCHUNK):
                cv = workp.tile([128, NCAND], F32, tag="cv")
                for w in range(NWIN):
                    w0 = w * WIN
                    wlen = min(WIN, NPER - w0)
                    ps = psp.tile([128, WIN], F32, tag="ps")
                    nt = (wlen + TW - 1) // TW
                    for ti in range(nt):
                        c0 = w0 + ti * TW
                        tw = min(TW, NPER - c0)
                        nc.tensor.matmul(
                            ps[:, ti * TW:ti * TW + tw],
                            lab_t[:, :, lc * 128:(lc + 1) * 128],
                            rhs_slice(c0, tw),
                            start=True, stop=True, perf_mode=DR)
                    nc.vector.max(out=cv[:, w * 8:(w + 1) * 8],
                                  in_=ps[:, :wlen])
                nc.gpsimd.dma_start(out=cand[lc * 128:(lc + 1) * 128, :],
                                    in_=cv[:])
    nc.finalize()
    return nc


def _rotation(labels):
    """Right singular basis of the label matrix, so the 6 dropped data
    channels align with the labels' least-energy directions."""
    _, _, Vt = np.linalg.svd(labels.astype(np.float64), full_matrices=True)
    return np.ascontiguousarray(Vt.T.astype(np.float32))  # (256, 256)


def _quantize_inputs(P, labels):
    """Build per-core fp8 operands with |p|^2 + index aux channels.

    All channel values are exact in e4m3 (DoubleRow pairs are
    magnitude-homogeneous so the reduced-precision in-cell pair-sum is
    exact); products land on a 0.5 grid
    (data), grid >= 2 (|p|^2) or 2^-13..2^-5 (index fields), so the fp32
    accumulation is bit-exact and the host can decode score + index.
    """
    V = _rotation(labels)
    Pr = P @ V                                                    # rotated
    Lr = labels @ V
    psq = (P.astype(np.float64) ** 2).sum(1).astype(np.float32)   # (100000,)
    ch = np.zeros((C, NCORES, NPER), np.float32)
    pq = np.clip(np.round(2.0 * Pr[:, :D]) * 0.5, -7.5, 7.5)      # (N, 250)
    ch[:D] = pq.T.reshape(D, NCORES, NPER)
    A = np.round(psq / 32.0)
    B = np.round(psq - 32.0 * A)
    ch[D] = A.reshape(NCORES, NPER)
    ch[D + 1] = B.reshape(NCORES, NPER)
    j = np.arange(NPER) % WIN
    ch[D + 2] = (((j >> 8) & 7) * 2.0 ** -5)[None, :]             # w=1
    ch[D + 3] = (((j >> 5) & 7) * 2.0 ** -8)[None, :]             # w=1
    ch[D + 4] = (((j >> 2) & 7) * 2.0 ** -9)[None, :]             # w=2^-2
    ch[D + 5] = ((j & 3) * 2.0 ** -9)[None, :]                    # w=2^-4
    rhs = np.ascontiguousarray(ch.transpose(1, 0, 2)).reshape(
        NCORES, 128, 2, NPER).astype(ml_dtypes.float8_e4m3)

    lch = np.zeros((C, SL), np.float32)
    lch[:D, :S] = np.clip(np.round(4.0 * Lr[:, :D]), -15, 15).T
    lch[D, :S] = -64.0
    lch[D + 1, :S] = -2.0
    lch[D + 2, :S] = 1.0
    lch[D + 3, :S] = 1.0
    lch[D + 4, :S] = 0.25
    lch[D + 5, :S] = 2.0 ** -4
    lhsT = np.ascontiguousarray(lch).reshape(128, 2, SL).astype(
        ml_dtypes.float8_e4m3)
    return rhs, lhsT


def _run_device(P, labels):
    """Returns candv (8, 896, 56) fp32 (score + encoded in-window index)."""
    global LAST_RESULT
    if "nc" not in _CACHE:
        _CACHE["nc"] = _build_nc()
    nc = _CACHE["nc"]

    rhs, lhsT = _quantize_inputs(P, labels)
    in_maps = []
    for c in range(NCORES):
        in_maps.append({
            "labT": lhsT,
            "patA": np.ascontiguousarray(rhs[c, :, :, :SPLIT1]),
            "patB": np.ascontiguousarray(rhs[c, :, :, SPLIT1:SPLIT2]),
            "patC": np.ascontiguousarray(rhs[c, :, :, SPLIT2:]),
        })
    res = run_bass_kernel_spmd(nc, in_maps, core_ids=list(range(NCORES)))
    LAST_RESULT = res
    return np.stack([np.asarray(r["cand"]) for r in res.results])


def _decode_candidates(candv):
    """(8, 896, 56) fp32 -> global patch ids (8, 896, 56)."""
    v = candv.astype(np.float64)
    s = np.floor(v * 2.0) * 0.5
    j = np.rint((v - s) * 8192.0).astype(np.int64)          # in-window index
    w = (np.arange(NCAND, dtype=np.int64) // 8) * WIN       # window base col
    col = j + w[None, None, :]
    core = (np.arange(NCORES, dtype=np.int64) * NPER)[:, None, None]
    gid = np.where(col < NPER, col + core, -1)
    return gid, s


def _topk_ctx_exact(labels_sl, gid, P, psq, k=9):
    """Union of per-core candidates -> exact rescore -> top-k -> ctx."""
    n = labels_sl.shape[0]
    g = gid[:, :n].transpose(1, 0, 2).reshape(n, -1)        # (n, 448)
    g_safe = np.where(g >= 0, g, 0)
    nb = P[g_safe]                                          # (n, 448, 256)
    d2 = (psq[g_safe] - 2.0 * np.einsum('nc,nkc->nk', labels_sl, nb,
                                        optimize=True)
          + (labels_sl * labels_sl).sum(-1, keepdims=True)).astype(np.float32)
    d2 = np.where(g >= 0, d2, np.float32(np.inf))
    sel = np.argsort(d2, axis=1, kind="stable")[:, :k]
    idx9 = np.take_along_axis(g_safe, sel, axis=1)
    nbrs = P[idx9]
    return nbrs.max(axis=1) - labels_sl


def _label_topk_ctx(labels, tbl, k):
    """Small exact label<-label aggregation (matches reference ordering)."""
    d2 = ((labels * labels).sum(-1, keepdims=True)
          - 2.0 * labels @ tbl.T + (tbl * tbl).sum(-1)[None, :]).astype(np.float32)
    idx = np.argsort(d2, axis=1, kind="stable")[:, :k]
    nbrs = tbl[idx]
    return nbrs.max(axis=1) - labels


def _layer_norm(x, g, b):
    mu = x.mean(-1, keepdims=True)
    var = x.var(-1, keepdims=True)
    return (x - mu) / np.sqrt(var + EPS) * g + b


def kernel(patch_emb, mood_emb, genre_emb, sub_emb,
           Wm_w, Wm_b, Wg_w, Wg_b, Ws_w, Ws_b,
           lnm_g, lnm_b, lng_g, lng_b, lns_g, lns_b):
    P = np.ascontiguousarray(np.asarray(patch_emb, np.float32))
    mood_e = np.asarray(mood_emb, np.float32)
    genre_e = np.asarray(genre_emb, np.float32)
    sub_e = np.asarray(sub_emb, np.float32)
    labels = np.concatenate([mood_e, genre_e, sub_e], 0)

    candv = _run_device(P, labels)
    gid, _ = _decode_candidates(candv)
    psq = (P.astype(np.float64) ** 2).sum(1).astype(np.float32)

    ctx_m = _topk_ctx_exact(mood_e, gid[:, 0:64], P, psq)
    mood = _layer_norm(mood_e + np.concatenate([mood_e, ctx_m], -1) @ np.asarray(Wm_w)
                       + np.asarray(Wm_b), np.asarray(lnm_g), np.asarray(lnm_b))

    ctx_gp = _topk_ctx_exact(genre_e, gid[:, 64:320], P, psq)
    ctx_gm = _label_topk_ctx(genre_e, mood.astype(np.float32), 4)
    genre = _layer_norm(genre_e + np.concatenate([genre_e, ctx_gp, ctx_gm], -1)
                        @ np.asarray(Wg_w) + np.asarray(Wg_b),
                        np.asarray(lng_g), np.asarray(lng_b))

    ctx_sp = _topk_ctx_exact(sub_e, gid[:, 320:832], P, psq)
    ctx_sm = _label_topk_ctx(sub_e, mood.astype(np.float32), 3)
    ctx_sg = _label_topk_ctx(sub_e, genre.astype(np.float32), 4)
    sub = _layer_norm(sub_e + np.concatenate([sub_e, ctx_sp, ctx_sm, ctx_sg], -1)
                      @ np.asarray(Ws_w) + np.asarray(Ws_b),
                      np.asarray(lns_g), np.asarray(lns_b))

    return np.concatenate([mood, genre, sub], 0).astype(np.float32)


# revision 10
# speedup vs baseline: 1.0251x; 1.0251x over previous
"""Trainium2 kernel for nn_HATGNN: hierarchical label<-patch kNN aggregation.

The 99.9%-of-FLOPs part (832x100000 squared-euclidean cdist + top-9
selection) runs on 8 NeuronCores, patch-sharded (12500 rows/core).

Device-side design (per core):
- Scores z2[s,j] ~= 4L.s @ p_j - 2|p_j|^2 are computed with one fp8e4m3 DoubleRow
  matmul per 512-column tile (K=256 channels in a single pass).
  Operands are quantized to coarse integer grids (labels: round(4l) ints,
  patches: 0.5-grid) so every product lands on a 0.5 grid and the PSUM
  accumulation is EXACT (score magnitude < 2^11, grid 2^-13: 24 bits).
- 6 of the 256 channels are repurposed as aux channels folding in
  (a) -2|p_j|^2 (2 channels, error <= 1 scaled) and (b) the column index
  within its 2048-wide selection window, encoded in mantissa bits
  2^-5..2^-13 (4 channels, exact).  A single fp32 score carries both.
- The ONLY selection work is one Max8 per 2048-wide (4 PSUM banks)
  window: 7 windows x 7 label chunks per core.  No FIND_INDEX8 pass, no
  |p|^2 rank-1 matmul, no index DMA.
- The host decodes (score, column) from the fp32 values, merges the
  8-core union (448 candidates/label), rescores candidates EXACTLY and
  takes the true top-9.  Quantization noise (sigma ~4 on a d^2 scale
  where the union-miss margin is ~40) only affects which candidates
  enter the union, not the final ordering.  The 6 dropped data channels
  are aligned with the label matrix's least-energy singular directions.
- The tiny 3-level MLP/LayerNorm pipeline (<=832 rows) runs in numpy.
"""
import numpy as np
import ml_dtypes

import concourse.bacc as bacc
import concourse.mybir as mybir
from concourse.tile import TileContext
from concourse.bass_utils import run_bass_kernel_spmd

NCORES = 8
NPER = 12500          # patches per core (no padding)
TW = 512              # matmul tile width (one PSUM bank)
WIN = 2048            # selection window (4 PSUM banks)
NWIN = 7              # 6 x 2048 + 1 x 212
SPLIT1 = 2048         # patch SBUF tiles: window 0 | tiles 4-12 | tiles 13-24
SPLIT2 = 6656
S = 832               # total labels (64 mood + 256 genre + 512 sub)
SL = 896              # padded to 7 x 128
NCHUNK = SL // 128    # 7 label chunks
C = 256
D = 250               # data channels (6 aux)
NCAND = NWIN * 8      # 56 candidates per label per core
EPS = 1e-5

F8 = mybir.dt.float8e4
F32 = mybir.dt.float32
PC_PAD = 5856         # pC SBUF tile cols (NPER-SPLIT2=5844 padded to %16==0
                      # for the DoubleRow ko-stride; last 12 cols never read)

_CACHE = {}
LAST_RESULT = None    # BassKernelResults of the most recent device run


def _build_nc():
    nc = bacc.Bacc()
    labT = nc.dram_tensor("labT", [128, 2, SL], F8, kind="ExternalInput")
    patA = nc.dram_tensor("patA", [128, 2, SPLIT1], F8, kind="ExternalInput")
    patB = nc.dram_tensor("patB", [128, 2, SPLIT2 - SPLIT1], F8,
                          kind="ExternalInput")
    patC = nc.dram_tensor("patC", [128, 2, NPER - SPLIT2], F8,
                          kind="ExternalInput")
    cand = nc.dram_tensor("cand", [SL, NCAND], F32, kind="ExternalOutput")

    DR = mybir.MatmulPerfMode.DoubleRow

    with TileContext(nc) as tc:
        with tc.tile_pool(name="big", bufs=1) as bigp, \
             tc.tile_pool(name="work", bufs=NCHUNK) as workp, \
             tc.tile_pool(name="ps", bufs=2, space="PSUM") as psp:
            lab_t = bigp.tile([128, 2, SL], F8, tag="lab")
            pA_t = bigp.tile([128, 2, SPLIT1], F8, tag="pA")
            pB_t = bigp.tile([128, 2, SPLIT2 - SPLIT1], F8, tag="pB")
            pC_t = bigp.tile([128, 2, PC_PAD], F8, tag="pC")
            nc.sync.dma_start(out=lab_t[:], in_=labT[:])
            nc.sync.dma_start(out=pA_t[:], in_=patA[:])
            nc.sync.dma_start(out=pB_t[:], in_=patB[:])
            nc.sync.dma_start(out=pC_t[:, :, :NPER - SPLIT2], in_=patC[:])

            def rhs_slice(c0, w):
                if c0 < SPLIT1:
                    return pA_t[:, :, c0:c0 + w]
                if c0 < SPLIT2:
                    return pB_t[:, :, c0 - SPLIT1:c0 - SPLIT1 + w]
                return pC_t[:, :, c0 - SPLIT2:c0 - SPLIT2 + w]

            for lc in range(NCHUNK):
                cv = workp.tile([128, NCAND], F32, tag="cv")
                for w in range(NWIN):
                    w0 = w * WIN
                    wlen = min(WIN, NPER - w0)
                    ps = psp.tile([128, WIN], F32, tag="ps")
                    nt = (wlen + TW - 1) // TW
                    for ti in range(nt):
                        c0 = w0 + ti * TW
                        tw = min(TW, NPER - c0)
                        nc.tensor.matmul(
                            ps[:, ti * TW:ti * TW + tw],
                            lab_t[:, :, lc * 128:(lc + 1) * 128],
                            rhs_slice(c0, tw),
                            start=True, stop=True, perf_mode=DR)
                    nc.vector.max(out=cv[:, w * 8:(w + 1) * 8],
                                  in_=ps[:, :wlen])
                nc.gpsimd.dma_start(out=cand[lc * 128:(lc + 1) * 128, :],
                                    in_=cv[:])
    nc.finalize()
    return nc


def _rotation(labels):
    """Right singular basis of the label matrix, so the 6 dropped data
    channels align with the labels' least-energy directions."""
    _, _, Vt = np.linalg.svd(labels.astype(np.float64), full_matrices=True)
    return np.ascontiguousarray(Vt.T.astype(np.float32))  # (256, 256)


def _quantize_inputs(P, labels):
    """Build per-core fp8 operands with |p|^2 + index aux channels.

    All channel values are exact in e4m3 (DoubleRow pairs are
    magnitude-homogeneous so the reduced-precision in-cell pair-sum is
    exact); products land on a 0.5 grid
    (data), grid >= 2 (|p|^2) or 2^-13..2^-5 (index fields), so the fp32
    accumulation is bit-exact and the host can decode score + index.
    """
    V = _rotation(labels)
    Pr = P @ V                                                    # rotated
    Lr = labels @ V
    psq = (P.astype(np.float64) ** 2).sum(1).astype(np.float32)   # (100000,)
    ch = np.zeros((C, NCORES, NPER), np.float32)
    pq = np.clip(np.round(2.0 * Pr[:, :D]) * 0.5, -7.5, 7.5)      # (N, 250)
    ch[:D] = pq.T.reshape(D, NCORES, NPER)
    A = np.round(psq / 32.0)
    B = np.round(psq - 32.0 * A)
    ch[D] = A.reshape(NCORES, NPER)
    ch[D + 1] = B.reshape(NCORES, NPER)
    j = np.arange(NPER) % WIN
    ch[D + 2] = (((j >> 8) & 7) * 2.0 ** -5)[None, :]             # w=1
    ch[D + 3] = (((j >> 5) & 7) * 2.0 ** -8)[None, :]             # w=1
    ch[D + 4] = (((j >> 2) & 7) * 2.0 ** -9)[None, :]             # w=2^-2
    ch[D + 5] = ((j & 3) * 2.0 ** -9)[None, :]                    # w=2^-4
    rhs = np.ascontiguousarray(ch.transpose(1, 0, 2)).reshape(
        NCORES, 128, 2, NPER).astype(ml_dtypes.float8_e4m3)

    lch = np.zeros((C, SL), np.float32)
    lch[:D, :S] = np.clip(np.round(4.0 * Lr[:, :D]), -15, 15).T
    lch[D, :S] = -64.0
    lch[D + 1, :S] = -2.0
    lch[D + 2, :S] = 1.0
    lch[D + 3, :S] = 1.0
    lch[D + 4, :S] = 0.25
    lch[D + 5, :S] = 2.0 ** -4
    lhsT = np.ascontiguousarray(lch).reshape(128, 2, SL).astype(
        ml_dtypes.float8_e4m3)
    return rhs, lhsT


def _run_device(P, labels):
    """Returns candv (8, 896, 56) fp32 (score + encoded in-window index)."""
    global LAST_RESULT
    if "nc" not in _CACHE:
        _CACHE["nc"] = _build_nc()
    nc = _CACHE["nc"]

    rhs, lhsT = _quantize_inputs(P, labels)
    in_maps = []
    for c in range(NCORES):
        in_maps.append({
            "labT": lhsT,
            "patA": np.ascontiguousarray(rhs[c, :, :, :SPLIT1]),
            "patB": np.ascontiguousarray(rhs[c, :, :, SPLIT1:SPLIT2]),
            "patC": np.ascontiguousarray(rhs[c, :, :, SPLIT2:]),
        })
    res = run_bass_kernel_spmd(nc, in_maps, core_ids=list(range(NCORES)))
    LAST_RESULT = res
    return np.stack([np.asarray(r["cand"]) for r in res.results])


def _decode_candidates(candv):
    """(8, 896, 56) fp32 -> global patch ids (8, 896, 56)."""
    v = candv.astype(np.float64)
    s = np.floor(v * 2.0) * 0.5
    j = np.rint((v - s) * 8192.0).astype(np.int64)          # in-window index
    w = (np.arange(NCAND, dtype=np.int64) // 8) * WIN       # window base col
    col = j + w[None, None, :]
    core = (np.arange(NCORES, dtype=np.int64) * NPER)[:, None, None]
    gid = np.where(col < NPER, col + core, -1)
    return gid, s


def _topk_ctx_exact(labels_sl, gid, P, psq, k=9):
    """Union of per-core candidates -> exact rescore -> top-k -> ctx."""
    n = labels_sl.shape[0]
    g = gid[:, :n].transpose(1, 0, 2).reshape(n, -1)        # (n, 448)
    g_safe = np.where(g >= 0, g, 0)
    nb = P[g_safe]                                          # (n, 448, 256)
    d2 = (psq[g_safe] - 2.0 * np.einsum('nc,nkc->nk', labels_sl, nb,
                                        optimize=True)
          + (labels_sl * labels_sl).sum(-1, keepdims=True)).astype(np.float32)
    d2 = np.where(g >= 0, d2, np.float32(np.inf))
    sel = np.argsort(d2, axis=1, kind="stable")[:, :k]
    idx9 = np.take_along_axis(g_safe, sel, axis=1)
    nbrs = P[idx9]
    return nbrs.max(axis=1) - labels_sl


def _label_topk_ctx(labels, tbl, k):
    """Small exact label<-label aggregation (matches reference ordering)."""
    d2 = ((labels * labels).sum(-1, keepdims=True)
          - 2.0 * labels @ tbl.T + (tbl * tbl).sum(-1)[None, :]).astype(np.float32)
    idx = np.argsort(d2, axis=1, kind="stable")[:, :k]
    nbrs = tbl[idx]
    return nbrs.max(axis=1) - labels


def _layer_norm(x, g, b):
    mu = x.mean(-1, keepdims=True)
    var = x.var(-1, keepdims=True)
    return (x - mu) / np.sqrt(var + EPS) * g + b


def kernel(patch_emb, mood_emb, genre_emb, sub_emb,
           Wm_w, Wm_b, Wg_w, Wg_b, Ws_w, Ws_b,
           lnm_g, lnm_b, lng_g, lng_b, lns_g, lns_b):
    P = np.ascontiguousarray(np.asarray(patch_emb, np.float32))
    mood_e = np.asarray(mood_emb, np.float32)
    genre_e = np.asarray(genre_emb, np.float32)
    sub_e = np.asarray(sub_emb, np.float32)
    labels = np.concatenate([mood_e, genre_e, sub_e], 0)

    candv = _run_device(P, labels)
    gid, _ = _decode_candidates(candv)
    psq = (P.astype(np.float64) ** 2).sum(1).astype(np.float32)

    ctx_m = _topk_ctx_exact(mood_e, gid[:, 0:64], P, psq)
    mood = _layer_norm(mood_e + np.concatenate([mood_e, ctx_m], -1) @ np.asarray(Wm_w)
                       + np.asarray(Wm_b), np.asarray(lnm_g), np.asarray(lnm_b))

    ctx_gp = _topk_ctx_exact(genre_e, gid[:, 64:320], P, psq)
    ctx_gm = _label_topk_ctx(genre_e, mood.astype(np.float32), 4)
    genre = _layer_norm(genre_e + np.concatenate([genre_e, ctx_gp, ctx_gm], -1)
                        @ np.asarray(Wg_w) + np.asarray(Wg_b),
                        np.asarray(lng_g), np.asarray(lng_b))

    ctx_sp = _topk_ctx_exact(sub_e, gid[:, 320:832], P, psq)
    ctx_sm = _label_topk_ctx(sub_e, mood.astype(np.float32), 3)
    ctx_sg = _label_topk_ctx(sub_e, genre.astype(np.float32), 4)
    sub = _layer_norm(sub_e + np.concatenate([sub_e, ctx_sp, ctx_sm, ctx_sg], -1)
                      @ np.asarray(Ws_w) + np.asarray(Ws_b),
                      np.asarray(lns_g), np.asarray(lns_b))

    return np.concatenate([mood, genre, sub], 0).astype(np.float32)
